# revision 2
# baseline (speedup 1.0000x reference)
"""Trainium2 Bass kernel for CustomConvWithExtra.

out = conv3x3(x, w_main) + b_main + extra, where extra collapses to a 3x3
border-class table T[b,c,clsh,clsw] (conv of a spatially-constant image).

Design (v2, targeting ~120us/core from 183us):
 - Data parallel: 1 batch image per NeuronCore (B=8 = 8 cores).
 - kw folded into the matmul contraction dim: ONE bf16 matmul per output
   row-pair [39,128]^T x [39,512] -> f32 PSUM bank [128,512].  Rows 0:36
   are (kw,d,ci); rows 36:39 are statics (indL, indR, ones) fusing bias +
   border-class terms.
 - NEW: only the 12 kw=1 rows are read from HBM (3.2MB vs 9.4MB).  Rows are
   stored PADDED (514 elems per pair segment, matching the padded image
   width), so the kw=0 / kw=2 duplicate partitions are pure +-1-element
   shifted copies with NO pair-boundary fixups: two SBUF->SBUF DMA copies
   per chunk build them.  HBM reads were latency-bound (~12GB/s per DMA
   engine vs ~26GB/s for streaming), so replacing 6.3MB of HBM reads with
   cheap SBUF->SBUF traffic relieves the loaded engines.
 - NEW: PSUM banks are single-port SRAM.  The old vector(288)+scalar(224)
   split drain of ONE bank made both engines fight over the port (~512ns/
   pair lockstep).  Now each bank is drained WHOLE by one engine,
   alternating vector(3):scalar(2), so the two engines stream different
   banks in parallel.
 - All DMA-visible data is bf16; PSUM stays f32; host casts the output
   back.  Rel err ~3.6e-3 vs the 2e-2 gate.
 - Fill + dup for chunk ch+1 are dispatched BEFORE chunk ch's output
   DMA in the gpsimd program order, so the (FIFO) SWDGE stream never puts
   a dup on the next chunk's critical path.
 - Output: DRAM laid out [chunk, 128, c*512] exactly as produced -> ONE
   contiguous SWDGE DMA per chunk (quartered for first/last chunk to cut
   ramp/tail).  Host un-permutes with numpy (free w.r.t. HW exec time).
"""

from contextlib import ExitStack

import ml_dtypes
import numpy as np

import concourse.bass as bass
import concourse.tile as tile
from concourse import bacc, mybir
from concourse.bass_utils import run_bass_kernel_spmd

# Problem shapes (hardcoded per contract)
B, CIN, H, W = 8, 3, 512, 512
COUT, E, KS = 64, 3, 3
NCORES = 8
KP = 39            # patch partitions: 36 = (kw,d,ci) + indL + indR + ones
C = 16             # row-pairs per chunk
PW = W + 2         # padded segment width (514): makes kw-shifts boundary-free
BF16 = mybir.dt.bfloat16
F32 = mybir.dt.float32
NPBF16 = ml_dtypes.bfloat16

_cache: dict = {}


def _build(h: int = H, w: int = W):
    pairs = h // 2
    c = min(C, pairs)
    nchunk = pairs // c
    assert pairs % c == 0
    cw = c * w       # free elements per chunk per partition (output layout)
    cw2 = c * PW     # padded free elements per chunk per partition (input)

    nc = bacc.Bacc("TRN2", target_bir_lowering=False, debug=False)
    xin = nc.dram_tensor("xin", [nchunk, 12, cw2], BF16, kind="ExternalInput").ap()
    wts = nc.dram_tensor("wts", [KP, 3 * 128], BF16, kind="ExternalInput").ap()
    stat = nc.dram_tensor("stat", [3, cw2], BF16, kind="ExternalInput").ap()
    # Output in BF16: halves the dominant 67 MB of HBM write traffic; the
    # PSUM->SBUF copies convert f32->bf16 for free and the host casts back.
    out = nc.dram_tensor("out", [nchunk, 128, cw], BF16, kind="ExternalOutput").ap()

    PBUFS = 6
    OBUFS = 6

    def fill(pt, ch):
        # 12 contiguous 16.4KB descriptors per chunk; rings alternate.
        base = ch * 12 * cw2
        if ch < 2:
            nc.sync.dma_start(
                pt[12:18, :], bass.AP(xin.tensor, base, [[cw2, 6], [1, cw2]])
            )
            nc.scalar.dma_start(
                pt[18:24, :],
                bass.AP(xin.tensor, base + 6 * cw2, [[cw2, 6], [1, cw2]]),
            )
        else:
            eng = (nc.sync, nc.scalar)[ch % 2]
            eng.dma_start(
                pt[12:24, :], bass.AP(xin.tensor, base, [[cw2, 12], [1, cw2]])
            )

    def dup(pt):
        # Build kw=0 / kw=2 partitions from the kw=1 rows by +-1-element
        # shifted SBUF->SBUF copies.  The 2-elem inter-pair gap (PW=514)
        # guarantees shifted reads never cross a pair boundary, so no
        # boundary fixups are needed.  kw0row[0] / kw2row[cw2-1] stay
        # unwritten and are never read (matmul window is [j*PW+1, j*PW+513)).
        nc.gpsimd.dma_start(pt[0:12, 1:cw2], pt[12:24, 0 : cw2 - 1])
        nc.gpsimd.dma_start(pt[24:36, 0 : cw2 - 1], pt[12:24, 1:cw2])

    with tile.TileContext(nc) as tc, ExitStack() as ctx:
        wpool = ctx.enter_context(tc.tile_pool(name="wpool", bufs=1))
        ppool = ctx.enter_context(tc.tile_pool(name="ppool", bufs=PBUFS))
        opool = ctx.enter_context(tc.tile_pool(name="opool", bufs=OBUFS))
        pspool = ctx.enter_context(tc.tile_pool(name="pspool", bufs=8, space="PSUM"))

        # One-time loads go on the gpsimd (output) ring — idle at t=0.
        # wts is pre-transposed on host to wtile's exact layout.
        wtile = wpool.tile([KP, 3 * 128], BF16)
        nc.gpsimd.dma_start(
            wtile[:, :], bass.AP(wts.tensor, 0, [[3 * 128, KP], [1, 3 * 128]])
        )

        # Patch buffers; static rows 36:39 loaded once per physical buffer.
        patch_tiles = []
        for s in range(PBUFS):
            pt = ppool.tile([KP, cw2], BF16, name=f"patch{s}", tag="patch")
            nc.gpsimd.dma_start(pt[36:39, :], stat[:, :])
            patch_tiles.append(pt)

        # Prologue: chunk 0's input is in flight before the loop.
        fill(patch_tiles[0], 0)
        dup(patch_tiles[0])

        for ch in range(nchunk):
            pt = patch_tiles[ch % PBUFS]
            # Dispatch NEXT chunk's fill + dup first: keeps the dup ahead of
            # this chunk's output DMA in the FIFO gpsimd stream.
            if ch + 1 < nchunk:
                ptn = patch_tiles[(ch + 1) % PBUFS]
                fill(ptn, ch + 1)
                dup(ptn)

            ob = opool.tile([128, cw], BF16, name="ob", tag="ob")
            # First/last chunk drain in four quarters: early quarters start
            # draining while later ones compute (shorter ramp/tail).
            halves = 4 if ch in (0, nchunk - 1) else 1
            for half in range(halves):
                j0 = half * c // halves
                j1 = (half + 1) * c // halves
                for j in range(j0, j1):
                    pairidx = ch * c + j
                    vrow = 0 if pairidx == 0 else (2 if pairidx == pairs - 1 else 1)
                    ps = pspool.tile([128, w], F32, name="ps", tag="ps")
                    nc.tensor.matmul(
                        ps[:, :],
                        wtile[:, vrow * 128 : (vrow + 1) * 128],
                        pt[:, j * PW + 1 : j * PW + 1 + w],
                        start=True,
                        stop=True,
                    )
                    # Whole-bank drains, alternating engines 3:2 — the two
                    # engines stream DIFFERENT banks in parallel (PSUM banks
                    # are single-ported; splitting one bank between engines
                    # serializes on the port).
                    if j % 5 < 3:
                        nc.vector.tensor_copy(
                            ob[:, j * w : (j + 1) * w], ps[:, :]
                        )
                    else:
                        nc.scalar.copy(ob[:, j * w : (j + 1) * w], ps[:, :])

                dst = bass.AP(
                    out.tensor,
                    ch * 128 * cw + j0 * w,
                    [[cw, 128], [1, (j1 - j0) * w]],
                )
                nc.gpsimd.dma_start(dst, ob[:, j0 * w : j1 * w])

    nc.compile()
    return nc


def _host_prep(x, v, wm, bm, we, be, h=H, w=W, c=C):
    """Per-core inputs: packed padded kw=1 row-planes, fused weights, statics."""
    Bb = x.shape[0]
    pairs = h // 2
    nchunk = pairs // c
    vr = v.reshape(Bb, COUT, E).astype(np.float64)

    # Border-class table for the "extra" conv of a spatially-constant image:
    # T[b,c,clsh,clsw] = sum of kernel taps landing inside + both biases.
    sets = {0: [1, 2], 1: [0, 1, 2], 2: [0, 1]}
    Mcl = np.zeros((COUT, E, 3, 3), np.float64)
    we64 = we.astype(np.float64)
    for ch_ in range(3):
        for cw_ in range(3):
            Mcl[:, :, ch_, cw_] = we64[:, :, sets[ch_], :][:, :, :, sets[cw_]].sum((2, 3))
    T = (
        np.einsum("bce,cehw->bchw", vr, Mcl)
        + bm.astype(np.float64)[None, :, None, None]
        + be.astype(np.float64)[None, :, None, None]
    )

    xp = np.pad(x, ((0, 0), (0, 0), (1, 1), (1, 1))).astype(np.float32)
    # xin[b, ch, d*3+ci, j*PW + t] = xp[b, ci, ch*2c + 2j + d, t], t in [0,514)
    xin = np.empty((Bb, nchunk, 12, c, PW), np.float32)
    for d in range(4):
        # rows d, d+2, ..., d+2*(pairs-1): [Bb, 3, pairs, PW]
        sl = xp[:, :, d : d + 2 * pairs - 1 : 2, :]
        xin[:, :, d * 3 : d * 3 + 3, :, :] = sl.reshape(
            Bb, CIN, nchunk, c, PW
        ).transpose(0, 2, 1, 3, 4)
    xin = xin.reshape(Bb, nchunk, 12, c * PW).astype(NPBF16)

    # vrow: 0 = pair (rows 0,1) classes (top,mid); 1 = interior; 2 = (mid,bot)
    pair_cls = {0: (0, 1), 1: (1, 1), 2: (1, 2)}
    wts = np.zeros((Bb, 3, KP, 128), np.float32)
    for b in range(Bb):
        for vrow in range(3):
            for pair in range(2):
                cols = slice(pair * 64, pair * 64 + 64)
                for kw in range(KS):
                    for d in range(4):
                        kh = d - pair
                        if 0 <= kh < KS:
                            for ci in range(CIN):
                                wts[b, vrow, kw * 12 + d * 3 + ci, cols] = wm[:, ci, kh, kw]
                cls = pair_cls[vrow][pair]
                wts[b, vrow, 36, cols] = T[b, :, cls, 0] - T[b, :, cls, 1]
                wts[b, vrow, 37, cols] = T[b, :, cls, 2] - T[b, :, cls, 1]
                wts[b, vrow, 38, cols] = T[b, :, cls, 1]

    # DRAM layout = wtile layout: wts2[b, k, v*128+m] = wts[b, v, k, m]
    wts2 = np.ascontiguousarray(wts.transpose(0, 2, 1, 3)).reshape(Bb, KP, 3 * 128)

    # Statics in the padded layout: matmul reads window [j*PW+1, j*PW+513)
    stat = np.zeros((3, c * PW), np.float32)
    stat[0, 1::PW] = 1.0           # output col 0 (left border class)
    stat[1, W::PW] = 1.0           # output col w-1 (right border class)
    stat[2, :] = 1.0               # ones row (base bias + interior class)
    return xin, wts2.astype(NPBF16), stat.astype(NPBF16)


def _unpack_out(o, h=H, w=W, c=C):
    """[nchunk, 128, c*w] -> [COUT, h, w]; partition = pair*64+co,
    free = j*w+x, row = ch*2c + 2j + pair."""
    nchunk = (h // 2) // c
    return (
        o.reshape(nchunk, 2, COUT, c, w)
        .transpose(2, 0, 3, 1, 4)
        .reshape(COUT, h, w)
    )


def kernel(**inputs) -> np.ndarray:
    x = np.ascontiguousarray(np.asarray(inputs["x"], np.float32))
    v = np.asarray(inputs["extra_inputs"], np.float32)
    wm = np.asarray(inputs["w_main"], np.float32)
    bm = np.asarray(inputs["b_main"], np.float32)
    we = np.asarray(inputs["w_extra"], np.float32)
    be = np.asarray(inputs["b_extra"], np.float32)

    xin, wts, stat = _host_prep(x, v, wm, bm, we, be)

    if "nc" not in _cache:
        _cache["nc"] = _build()
    nc = _cache["nc"]

    in_maps = [{"xin": xin[b], "wts": wts[b], "stat": stat} for b in range(B)]
    res = run_bass_kernel_spmd(nc, in_maps, list(range(NCORES)))
    return np.stack(
        [_unpack_out(res.results[b]["out"]) for b in range(B)]
    ).astype(np.float32)


# revision 9
# speedup vs baseline: 1.2791x; 1.2791x over previous
"""Trainium2 Bass kernel for CustomConvWithExtra.

out = conv3x3(x, w_main) + b_main + extra, where extra collapses to a 3x3
border-class table T[b,c,clsh,clsw] (conv of a spatially-constant image).

Design v3 (from 183us baseline; DMA packet-latency is the binding law):
 - Data parallel: 1 batch image per NeuronCore (B=8 = 8 cores).
 - Empirical DMA law (measured): ANY packet that WRITES SBUF costs ~1.05us
   flat (HBM round-trip latency, no pipelining, packet <= 16KB); only
   SBUF->HBM writes stream at ~26GB/s/engine.  So the input path must
   minimize SBUF-write packet count and the kw=0/2 duplicate planes must
   NOT go through DMA at all.
 - Supertile = 32 output row-pairs; 8 supertiles.  Patch tile [88, 8192]
   bf16: each kw plane holds 24 rows = (d,ci) x (g=half), each row 16
   pair-segments of 512 = 16KB -> fill is 24 descriptors of EXACTLY 16KB
   (one full packet each; 3.2MB total input = ~193 packets).
 - kw planes at 32-aligned partition bases (0/32/64) so the kw0/kw2
   duplicates are built by VECTOR-engine partition-crossing shifted
   copies (nch=24 quadrant moves, DVE 4x bf16 copy).  Pair-boundary
   columns (always zero: the image's L/R padding) are fixed by tiny
   strided memsets; gap rows 27-31/56-63 are zero-weighted, memset once.
 - Statics (indL, indR, ones) at partitions 24:27 fuse bias+border terms.
 - PSUM: 4 x [128,1024] double-bank tiles; two matmuls fill the halves,
   ONE wide drain (vector or scalar) empties both banks - amortizes the
   ~120-170cyc fixed cost and keeps the two engines on DIFFERENT banks
   (PSUM banks are single-ported; v1 split one bank between engines and
   serialized on the port at 512ns/pair).
 - All DMA-visible data is bf16; PSUM stays f32; host casts the output
   back.  Rel err ~3.6e-3 vs the 2e-2 gate.
 - Output: DRAM laid out [chunk=16pairs, 128, 8192] exactly as produced
   -> ONE contiguous SWDGE DMA per chunk (quartered for first/last chunk
   to cut ramp/tail).  Host un-permutes with numpy.
"""

from contextlib import ExitStack

import ml_dtypes
import numpy as np

import concourse.bass as bass
import concourse.tile as tile
from concourse import bacc, mybir
from concourse.bass_utils import run_bass_kernel_spmd

# Problem shapes (hardcoded per contract)
B, CIN, H, W = 8, 3, 512, 512
COUT, E, KS = 64, 3, 3
NCORES = 8
KP = 88            # contraction: kw0 0:24, statics 24:27, 0s 27:32,
                   #              kw1 32:56, 0s 56:64, kw2 64:88
NST = 8            # supertiles
SW = 16 * W        # free elems per partition per supertile (8192 = 16KB bf16)
BF16 = mybir.dt.bfloat16
F32 = mybir.dt.float32
NPBF16 = ml_dtypes.bfloat16

_cache: dict = {}


def _build():
    nchunk = NST * 2          # 16 chunks of 16 pairs (output granularity)
    cw = SW                   # 8192

    nc = bacc.Bacc("TRN2", target_bir_lowering=False, debug=False)
    xin = nc.dram_tensor("xin", [NST, 24, SW], BF16, kind="ExternalInput").ap()
    wts = nc.dram_tensor("wts", [KP, 4 * 128], BF16, kind="ExternalInput").ap()
    # statics (indL, indR, ones) + 5 zero rows: lands on partitions 24:32
    stat = nc.dram_tensor("stat", [8, SW], BF16, kind="ExternalInput").ap()
    out = nc.dram_tensor("out", [nchunk, 128, cw], BF16, kind="ExternalOutput").ap()

    PBUFS = 4
    OBUFS = 6
    # drains handled by vector for these (g*8+seg//2) indices, scalar else;
    # vector also carries the dups, so scalar takes the bigger drain share.
    VDRAIN = {1, 4, 7, 10, 13}

    with tile.TileContext(nc) as tc, ExitStack() as ctx:
        wpool = ctx.enter_context(tc.tile_pool(name="wpool", bufs=1))
        ppool = ctx.enter_context(tc.tile_pool(name="ppool", bufs=PBUFS))
        opool = ctx.enter_context(tc.tile_pool(name="opool", bufs=OBUFS))
        pspool = ctx.enter_context(tc.tile_pool(name="pspool", bufs=4, space="PSUM"))

        wtile = wpool.tile([KP, 4 * 128], BF16)
        nc.gpsimd.dma_start(
            wtile[:, :], bass.AP(wts.tensor, 0, [[4 * 128, KP], [1, 4 * 128]])
        )

        patch_tiles = []
        for s in range(PBUFS):
            pt = ppool.tile([KP, SW], BF16, name=f"patch{s}", tag="patch")
            patch_tiles.append(pt)

        def init_buf(s):
            # One-time per physical buffer.  Engine ops may not cross a
            # 32-partition quadrant from an unaligned start, so: the
            # [32:64) memset zeroes gap rows 56:64 (fills overwrite 32:56),
            # and the statics DMA brings rows 24:32 (3 statics + 5 zeros).
            pt = patch_tiles[s]
            nc.vector.memset(pt[32:64, :], 0.0)
            nc.vector.memset(pt[0:24, 0:1], 0.0)
            nc.vector.memset(pt[64:88, SW - 1 : SW], 0.0)
            nc.gpsimd.dma_start(pt[24:32, :], stat[:, :])

        def fill(st):
            pt = patch_tiles[st % PBUFS]
            base = st * 24 * SW
            if st == 0:
                nc.sync.dma_start(
                    pt[32:44, :], bass.AP(xin.tensor, base, [[SW, 12], [1, SW]])
                )
                nc.scalar.dma_start(
                    pt[44:56, :],
                    bass.AP(xin.tensor, base + 12 * SW, [[SW, 12], [1, SW]]),
                )
            else:
                eng = (nc.sync, nc.scalar)[st % 2]
                eng.dma_start(
                    pt[32:56, :], bass.AP(xin.tensor, base, [[SW, 24], [1, SW]])
                )

        def dup(st):
            # Build kw0 (= kw1 shifted +1 elem) and kw2 (= kw1 shifted -1)
            # on the vector engine; even free-dim splits keep DVE 4x mode.
            pt = patch_tiles[st % PBUFS]
            h = SW // 2  # 4096
            nc.vector.tensor_copy(pt[0:24, 1 : h + 1], pt[32:56, 0:h])
            nc.vector.tensor_copy(pt[0:24, h + 1 : SW - 1], pt[32:56, h : SW - 2])
            nc.vector.tensor_copy(pt[0:24, SW - 1 : SW], pt[32:56, SW - 2 : SW - 1])
            nc.vector.tensor_copy(pt[64:88, 0:h], pt[32:56, 1 : h + 1])
            nc.vector.tensor_copy(pt[64:88, h : SW - 2], pt[32:56, h + 1 : SW - 1])
            nc.vector.tensor_copy(pt[64:88, SW - 2 : SW - 1], pt[32:56, SW - 1 : SW])
            # Pair-boundary columns are the image's L/R zero padding; the
            # shifted copies put the neighbour pair's edge there - zero them.
            nc.vector.memset(pt[0:24, W :: W], 0.0)
            nc.vector.memset(pt[64:88, W - 1 : SW - W : W], 0.0)

        init_buf(0)
        init_buf(1)
        fill(0)
        fill(1)
        # PE warm-up: ~12 back-to-back matmuls (~5us of continuous PE
        # activity) flip the HAM clock gate from 4/8 (1.2GHz) to 8/8
        # (2.4GHz) while the first fill + dup are still in flight.  The
        # steady-state MM stream (short gaps) then keeps it warm; without
        # this the whole kernel ran at 427ns/MM = the cold issue rate.
        ps_warm = pspool.tile([128, 2 * W], F32, name="ps", tag="ps")
        for _ in range(12):
            nc.tensor.matmul(
                ps_warm[:, 0:W],
                wtile[:, 0:128],
                wtile[:, 0:512],
                start=True,
                stop=True,
            )
        dup(0)

        for st in range(NST):
            pt = patch_tiles[st % PBUFS]
            if st + 2 < NST:
                if st + 2 < PBUFS:
                    init_buf(st + 2)
                fill(st + 2)

            for g in range(2):
                ch = st * 2 + g
                ob = opool.tile([128, cw], BF16, name="ob", tag="ob")
                quarters = 4 if ch in (0, nchunk - 1) else 1
                for q in range(quarters):
                    s0 = q * 16 // quarters
                    s1 = (q + 1) * 16 // quarters
                    for seg in range(s0, s1):
                        pairidx = st * 32 + g * 16 + seg
                        var = (
                            0
                            if pairidx == 0
                            else (3 if pairidx == 255 else (1 + g))
                        )
                        if seg % 2 == 0:
                            ps2 = pspool.tile([128, 2 * W], F32, name="ps", tag="ps")
                        half = seg % 2
                        nc.tensor.matmul(
                            ps2[:, half * W : (half + 1) * W],
                            wtile[:, var * 128 : (var + 1) * 128],
                            pt[:, seg * W : (seg + 1) * W],
                            start=True,
                            stop=True,
                        )
                        if seg % 2 == 1:
                            dst = ob[:, (seg - 1) * W : (seg + 1) * W]
                            if (g * 8 + seg // 2) in VDRAIN:
                                nc.vector.tensor_copy(dst, ps2[:, :])
                            else:
                                nc.scalar.copy(dst, ps2[:, :])

                    dma_dst = bass.AP(
                        out.tensor,
                        ch * 128 * cw + s0 * W,
                        [[cw, 128], [1, (s1 - s0) * W]],
                    )
                    nc.gpsimd.dma_start(dma_dst, ob[:, s0 * W : s1 * W])

                if g == 0 and st + 1 < NST:
                    # Mid-supertile: next supertile's dup sits between this
                    # supertile's two drain batches in the vector FIFO.
                    dup(st + 1)

    nc.compile()
    return nc


def _host_prep(x, v, wm, bm, we, be):
    """Per-core inputs: packed kw=1 row-planes (supertile layout), fused
    weights (4 stationary variants), statics."""
    Bb = x.shape[0]
    vr = v.reshape(Bb, COUT, E).astype(np.float64)

    # Border-class table for the "extra" conv of a spatially-constant image:
    # T[b,c,clsh,clsw] = sum of kernel taps landing inside + both biases.
    sets = {0: [1, 2], 1: [0, 1, 2], 2: [0, 1]}
    Mcl = np.zeros((COUT, E, 3, 3), np.float64)
    we64 = we.astype(np.float64)
    for ch_ in range(3):
        for cw_ in range(3):
            Mcl[:, :, ch_, cw_] = we64[:, :, sets[ch_], :][:, :, :, sets[cw_]].sum((2, 3))
    T = (
        np.einsum("bce,cehw->bchw", vr, Mcl)
        + bm.astype(np.float64)[None, :, None, None]
        + be.astype(np.float64)[None, :, None, None]
    )

    # xin[b, st, (d*3+ci)*2+g, seg*512+y] = xr[b, ci, 2*(st*32+g*16+seg)+d, y]
    xr = np.pad(x, ((0, 0), (0, 0), (1, 1), (0, 0))).astype(np.float32)
    xin5 = np.empty((Bb, NST, 12, 2, 16, W), np.float32)
    for d in range(4):
        sl = xr[:, :, d : d + 511 : 2, :]  # rows d, d+2, ..., d+510 -> 256
        xin5[:, :, d * 3 : (d + 1) * 3] = sl.reshape(
            Bb, CIN, NST, 2, 16, W
        ).transpose(0, 2, 1, 3, 4, 5)
    xin = xin5.reshape(Bb, NST, 24, SW).astype(NPBF16)

    # Stationary variants: (vrow, g) in [(0,0),(1,0),(1,1),(2,1)].
    # vrow: 0 = pair 0 (rows top,mid); 1 = interior; 2 = last pair (mid,bot)
    pair_cls = {0: (0, 1), 1: (1, 1), 2: (1, 2)}
    plane_base = {0: 0, 1: 32, 2: 64}
    var_map = [(0, 0), (1, 0), (1, 1), (2, 1)]
    wts = np.zeros((Bb, 4, KP, 128), np.float32)
    for b in range(Bb):
        for var, (vrow, gsel) in enumerate(var_map):
            for p in range(2):
                cols = slice(p * 64, p * 64 + 64)
                for kw in range(KS):
                    for d in range(4):
                        kh = d - p
                        if 0 <= kh < KS:
                            for ci in range(CIN):
                                k = plane_base[kw] + (d * 3 + ci) * 2 + gsel
                                wts[b, var, k, cols] = wm[:, ci, kh, kw]
                cls = pair_cls[vrow][p]
                wts[b, var, 24, cols] = T[b, :, cls, 0] - T[b, :, cls, 1]
                wts[b, var, 25, cols] = T[b, :, cls, 2] - T[b, :, cls, 1]
                wts[b, var, 26, cols] = T[b, :, cls, 1]

    # DRAM layout = wtile layout: wts2[b, k, var*128+m] = wts[b, var, k, m]
    wts2 = np.ascontiguousarray(wts.transpose(0, 2, 1, 3)).reshape(Bb, KP, 4 * 128)

    stat = np.zeros((8, SW), np.float32)
    stat[0, 0::W] = 1.0            # output col 0 (left border class)
    stat[1, W - 1 :: W] = 1.0      # output col w-1 (right border class)
    stat[2, :] = 1.0               # ones row (base bias + interior class)
    return xin, wts2.astype(NPBF16), stat.astype(NPBF16)


def _unpack_out(o, h=H, w=W, c=16):
    """[nchunk, 128, c*w] -> [COUT, h, w]; partition = p*64+co,
    free = seg*w+x, row = ch*2c + 2*seg + p."""
    nchunk = (h // 2) // c
    return (
        o.reshape(nchunk, 2, COUT, c, w)
        .transpose(2, 0, 3, 1, 4)
        .reshape(COUT, h, w)
    )


def kernel(**inputs) -> np.ndarray:
    x = np.ascontiguousarray(np.asarray(inputs["x"], np.float32))
    v = np.asarray(inputs["extra_inputs"], np.float32)
    wm = np.asarray(inputs["w_main"], np.float32)
    bm = np.asarray(inputs["b_main"], np.float32)
    we = np.asarray(inputs["w_extra"], np.float32)
    be = np.asarray(inputs["b_extra"], np.float32)

    xin, wts, stat = _host_prep(x, v, wm, bm, we, be)

    if "nc" not in _cache:
        _cache["nc"] = _build()
    nc = _cache["nc"]

    in_maps = [{"xin": xin[b], "wts": wts[b], "stat": stat} for b in range(B)]
    res = run_bass_kernel_spmd(nc, in_maps, list(range(NCORES)))
    return np.stack(
        [_unpack_out(res.results[b]["out"]) for b in range(B)]
    ).astype(np.float32)


# revision 13
# speedup vs baseline: 1.4039x; 1.0975x over previous
"""Trainium2 Bass kernel for CustomConvWithExtra.

out = conv3x3(x, w_main) + b_main + extra, where extra collapses to a 3x3
border-class table T[b,c,clsh,clsw] (conv of a spatially-constant image).

Design v3 (from 183us baseline; DMA packet-latency is the binding law):
 - Data parallel: 1 batch image per NeuronCore (B=8 = 8 cores).
 - Empirical DMA law (measured): ANY packet that WRITES SBUF costs ~1.05us
   flat (HBM round-trip latency, no pipelining, packet <= 16KB); only
   SBUF->HBM writes stream at ~26GB/s/engine.  So the input path must
   minimize SBUF-write packet count and the kw=0/2 duplicate planes must
   NOT go through DMA at all.
 - Supertile = 32 output row-pairs; 8 supertiles.  Patch tile [88, 8192]
   bf16: each kw plane holds 24 rows = (d,ci) x (g=half), each row 16
   pair-segments of 512 = 16KB -> fill is 24 descriptors of EXACTLY 16KB
   (one full packet each; 3.2MB total input = ~193 packets).
 - kw planes at 32-aligned partition bases (0/32/64) so the kw0/kw2
   duplicates are built by VECTOR-engine partition-crossing shifted
   copies (nch=24 quadrant moves, DVE 4x bf16 copy).  Pair-boundary
   columns (always zero: the image's L/R padding) are fixed by tiny
   strided memsets; gap rows 27-31/56-63 are zero-weighted, memset once.
 - Statics (indL, indR, ones) at partitions 24:27 fuse bias+border terms.
 - PSUM: 4 x [128,1024] double-bank tiles; two matmuls fill the halves,
   ONE wide drain (vector or scalar) empties both banks - amortizes the
   ~120-170cyc fixed cost and keeps the two engines on DIFFERENT banks
   (PSUM banks are single-ported; v1 split one bank between engines and
   serialized on the port at 512ns/pair).
 - All DMA-visible data is bf16; PSUM stays f32; host casts the output
   back.  Rel err ~3.6e-3 vs the 2e-2 gate.
 - Output: DRAM laid out [chunk=16pairs, 128, 8192] exactly as produced
   -> ONE contiguous SWDGE DMA per chunk (quartered for first/last chunk
   to cut ramp/tail).  Host un-permutes with numpy.
"""

from contextlib import ExitStack

import ml_dtypes
import numpy as np

import concourse.bass as bass
import concourse.tile as tile
from concourse import bacc, mybir
from concourse.bass_utils import run_bass_kernel_spmd

# Problem shapes (hardcoded per contract)
B, CIN, H, W = 8, 3, 512, 512
COUT, E, KS = 64, 3, 3
NCORES = 8
KP = 88            # contraction: kw0 0:24, statics 24:27, 0s 27:32,
                   #              kw1 32:56, 0s 56:64, kw2 64:88
NST = 8            # supertiles
SW = 16 * W        # free elems per partition per supertile (8192 = 16KB bf16)
BF16 = mybir.dt.bfloat16
F32 = mybir.dt.float32
NPBF16 = ml_dtypes.bfloat16

_cache: dict = {}


def _build():
    nchunk = NST * 2          # 16 chunks of 16 pairs (output granularity)
    cw = SW                   # 8192

    nc = bacc.Bacc("TRN2", target_bir_lowering=False, debug=False)
    # rows 0:24 = kw1 data; rows 24:32 = zeros, read only by each buffer's
    # FIRST fill to initialize the zero-weighted gap partitions 56:64
    # (DVE memsets of 8192 elems run at 1x = ~7us each - far too slow).
    xin = nc.dram_tensor("xin", [NST, 32, SW], BF16, kind="ExternalInput").ap()
    wts = nc.dram_tensor("wts", [KP, 4 * 128], BF16, kind="ExternalInput").ap()
    # statics (indL, indR, ones) + 5 zero rows: lands on partitions 24:32
    stat = nc.dram_tensor("stat", [8, SW], BF16, kind="ExternalInput").ap()
    out = nc.dram_tensor("out", [nchunk, 128, cw], BF16, kind="ExternalOutput").ap()

    PBUFS = 4
    OBUFS = 6
    # drains handled by vector for these (g*8+seg//2) indices, scalar else;
    # vector also carries the dups, so scalar takes the bigger drain share.
    VDRAIN = {1, 4, 7, 10, 13}

    with tile.TileContext(nc) as tc, ExitStack() as ctx:
        wpool = ctx.enter_context(tc.tile_pool(name="wpool", bufs=1))
        ppool = ctx.enter_context(tc.tile_pool(name="ppool", bufs=PBUFS))
        opool = ctx.enter_context(tc.tile_pool(name="opool", bufs=OBUFS))
        pspool = ctx.enter_context(tc.tile_pool(name="pspool", bufs=4, space="PSUM"))

        wtile = wpool.tile([KP, 4 * 128], BF16)
        nc.gpsimd.dma_start(
            wtile[:, :], bass.AP(wts.tensor, 0, [[4 * 128, KP], [1, 4 * 128]])
        )

        patch_tiles = []
        for s in range(PBUFS):
            pt = ppool.tile([KP, SW], BF16, name=f"patch{s}", tag="patch")
            patch_tiles.append(pt)

        def init_buf(s):
            # One-time per physical buffer: the two never-written shift
            # edges, and the statics DMA (rows 24:32 = 3 statics + 5 zeros).
            pt = patch_tiles[s]
            nc.vector.memset(pt[0:24, 0:1], 0.0)
            nc.vector.memset(pt[64:88, SW - 1 : SW], 0.0)
            nc.gpsimd.dma_start(pt[24:32, :], stat[:, :])

        def fill(st):
            # A buffer's first fill loads 32 rows (data + embedded zeros
            # for gap partitions 56:64); later fills reuse the zeros and
            # load only the 24 data rows.
            pt = patch_tiles[st % PBUFS]
            base = st * 32 * SW
            nrow = 32 if st < PBUFS else 24
            if st == 0:
                nc.sync.dma_start(
                    pt[32:48, :], bass.AP(xin.tensor, base, [[SW, 16], [1, SW]])
                )
                nc.scalar.dma_start(
                    pt[48:64, :],
                    bass.AP(xin.tensor, base + 16 * SW, [[SW, 16], [1, SW]]),
                )
            else:
                eng = (nc.sync, nc.scalar)[st % 2]
                eng.dma_start(
                    pt[32 : 32 + nrow, :],
                    bass.AP(xin.tensor, base, [[SW, nrow], [1, SW]]),
                )

        def dup(st):
            # Build kw0 (= kw1 shifted +1 elem) and kw2 (= kw1 shifted -1)
            # on the vector engine; even free-dim splits keep DVE 4x mode.
            pt = patch_tiles[st % PBUFS]
            h = SW // 2  # 4096
            nc.vector.tensor_copy(pt[0:24, 1 : h + 1], pt[32:56, 0:h])
            nc.vector.tensor_copy(pt[0:24, h + 1 : SW - 1], pt[32:56, h : SW - 2])
            nc.vector.tensor_copy(pt[0:24, SW - 1 : SW], pt[32:56, SW - 2 : SW - 1])
            nc.vector.tensor_copy(pt[64:88, 0:h], pt[32:56, 1 : h + 1])
            nc.vector.tensor_copy(pt[64:88, h : SW - 2], pt[32:56, h + 1 : SW - 1])
            nc.vector.tensor_copy(pt[64:88, SW - 2 : SW - 1], pt[32:56, SW - 1 : SW])
            # Pair-boundary columns are the image's L/R zero padding; the
            # shifted copies put the neighbour pair's edge there - zero them.
            nc.vector.memset(pt[0:24, W :: W], 0.0)
            nc.vector.memset(pt[64:88, W - 1 : SW - W : W], 0.0)

        init_buf(0)
        init_buf(1)
        fill(0)
        fill(1)
        # PE warm-up: 8 back-to-back matmuls (~3.4us of continuous PE
        # activity) to flip the HAM clock gate from 4/8 (1.2GHz) to 8/8
        # (2.4GHz); dup(0) completes before the burst ends so the real MM
        # stream follows with no gap (a gap > ~3.4us re-throttles).
        ps_warm = pspool.tile([128, 2 * W], F32, name="ps", tag="ps")
        for _ in range(8):
            nc.tensor.matmul(
                ps_warm[:, 0:W],
                wtile[:, 0:128],
                wtile[:, 0:512],
                start=True,
                stop=True,
            )
        dup(0)

        for st in range(NST):
            pt = patch_tiles[st % PBUFS]
            if st + 2 < NST:
                if st + 2 < PBUFS:
                    init_buf(st + 2)
                fill(st + 2)

            for g in range(2):
                ch = st * 2 + g
                ob = opool.tile([128, cw], BF16, name="ob", tag="ob")
                quarters = 4 if ch in (0, nchunk - 1) else 1
                for q in range(quarters):
                    s0 = q * 16 // quarters
                    s1 = (q + 1) * 16 // quarters
                    for seg in range(s0, s1):
                        pairidx = st * 32 + g * 16 + seg
                        var = (
                            0
                            if pairidx == 0
                            else (3 if pairidx == 255 else (1 + g))
                        )
                        if seg % 2 == 0:
                            ps2 = pspool.tile([128, 2 * W], F32, name="ps", tag="ps")
                        half = seg % 2
                        nc.tensor.matmul(
                            ps2[:, half * W : (half + 1) * W],
                            wtile[:, var * 128 : (var + 1) * 128],
                            pt[:, seg * W : (seg + 1) * W],
                            start=True,
                            stop=True,
                        )
                        if seg % 2 == 1:
                            dst = ob[:, (seg - 1) * W : (seg + 1) * W]
                            if (g * 8 + seg // 2) in VDRAIN:
                                nc.vector.tensor_copy(dst, ps2[:, :])
                            else:
                                nc.scalar.copy(dst, ps2[:, :])

                    dma_dst = bass.AP(
                        out.tensor,
                        ch * 128 * cw + s0 * W,
                        [[cw, 128], [1, (s1 - s0) * W]],
                    )
                    nc.gpsimd.dma_start(dma_dst, ob[:, s0 * W : s1 * W])

                if g == 0 and st + 1 < NST:
                    # Mid-supertile: next supertile's dup sits between this
                    # supertile's two drain batches in the vector FIFO.
                    dup(st + 1)

    nc.compile()
    return nc


def _host_prep(x, v, wm, bm, we, be):
    """Per-core inputs: packed kw=1 row-planes (supertile layout), fused
    weights (4 stationary variants), statics."""
    Bb = x.shape[0]
    vr = v.reshape(Bb, COUT, E).astype(np.float64)

    # Border-class table for the "extra" conv of a spatially-constant image:
    # T[b,c,clsh,clsw] = sum of kernel taps landing inside + both biases.
    sets = {0: [1, 2], 1: [0, 1, 2], 2: [0, 1]}
    Mcl = np.zeros((COUT, E, 3, 3), np.float64)
    we64 = we.astype(np.float64)
    for ch_ in range(3):
        for cw_ in range(3):
            Mcl[:, :, ch_, cw_] = we64[:, :, sets[ch_], :][:, :, :, sets[cw_]].sum((2, 3))
    T = (
        np.einsum("bce,cehw->bchw", vr, Mcl)
        + bm.astype(np.float64)[None, :, None, None]
        + be.astype(np.float64)[None, :, None, None]
    )

    # xin[b, st, (d*3+ci)*2+g, seg*512+y] = xr[b, ci, 2*(st*32+g*16+seg)+d, y]
    xr = np.pad(x, ((0, 0), (0, 0), (1, 1), (0, 0))).astype(np.float32)
    xin5 = np.zeros((Bb, NST, 16, 2, 16, W), np.float32)
    for d in range(4):
        sl = xr[:, :, d : d + 511 : 2, :]  # rows d, d+2, ..., d+510 -> 256
        xin5[:, :, d * 3 : (d + 1) * 3] = sl.reshape(
            Bb, CIN, NST, 2, 16, W
        ).transpose(0, 2, 1, 3, 4, 5)
    xin = xin5.reshape(Bb, NST, 32, SW).astype(NPBF16)

    # Stationary variants: (vrow, g) in [(0,0),(1,0),(1,1),(2,1)].
    # vrow: 0 = pair 0 (rows top,mid); 1 = interior; 2 = last pair (mid,bot)
    pair_cls = {0: (0, 1), 1: (1, 1), 2: (1, 2)}
    plane_base = {0: 0, 1: 32, 2: 64}
    var_map = [(0, 0), (1, 0), (1, 1), (2, 1)]
    wts = np.zeros((Bb, 4, KP, 128), np.float32)
    for b in range(Bb):
        for var, (vrow, gsel) in enumerate(var_map):
            for p in range(2):
                cols = slice(p * 64, p * 64 + 64)
                for kw in range(KS):
                    for d in range(4):
                        kh = d - p
                        if 0 <= kh < KS:
                            for ci in range(CIN):
                                k = plane_base[kw] + (d * 3 + ci) * 2 + gsel
                                wts[b, var, k, cols] = wm[:, ci, kh, kw]
                cls = pair_cls[vrow][p]
                wts[b, var, 24, cols] = T[b, :, cls, 0] - T[b, :, cls, 1]
                wts[b, var, 25, cols] = T[b, :, cls, 2] - T[b, :, cls, 1]
                wts[b, var, 26, cols] = T[b, :, cls, 1]

    # DRAM layout = wtile layout: wts2[b, k, var*128+m] = wts[b, var, k, m]
    wts2 = np.ascontiguousarray(wts.transpose(0, 2, 1, 3)).reshape(Bb, KP, 4 * 128)

    stat = np.zeros((8, SW), np.float32)
    stat[0, 0::W] = 1.0            # output col 0 (left border class)
    stat[1, W - 1 :: W] = 1.0      # output col w-1 (right border class)
    stat[2, :] = 1.0               # ones row (base bias + interior class)
    return xin, wts2.astype(NPBF16), stat.astype(NPBF16)


def _unpack_out(o, h=H, w=W, c=16):
    """[nchunk, 128, c*w] -> [COUT, h, w]; partition = p*64+co,
    free = seg*w+x, row = ch*2c + 2*seg + p."""
    nchunk = (h // 2) // c
    return (
        o.reshape(nchunk, 2, COUT, c, w)
        .transpose(2, 0, 3, 1, 4)
        .reshape(COUT, h, w)
    )


def kernel(**inputs) -> np.ndarray:
    x = np.ascontiguousarray(np.asarray(inputs["x"], np.float32))
    v = np.asarray(inputs["extra_inputs"], np.float32)
    wm = np.asarray(inputs["w_main"], np.float32)
    bm = np.asarray(inputs["b_main"], np.float32)
    we = np.asarray(inputs["w_extra"], np.float32)
    be = np.asarray(inputs["b_extra"], np.float32)

    xin, wts, stat = _host_prep(x, v, wm, bm, we, be)

    if "nc" not in _cache:
        _cache["nc"] = _build()
    nc = _cache["nc"]

    in_maps = [{"xin": xin[b], "wts": wts[b], "stat": stat} for b in range(B)]
    res = run_bass_kernel_spmd(nc, in_maps, list(range(NCORES)))
    return np.stack(
        [_unpack_out(res.results[b]["out"]) for b in range(B)]
    ).astype(np.float32)
